# revision 1
# baseline (speedup 1.0000x reference)
"""Trainium2 Bass kernel: causal multi-head attention block (B=2, S=2048, D=4096,
32 heads x 128 head_dim, fp32, interleaved RoPE) tensor-parallel over heads on
8 NeuronCores, with a per-batch AllToAll to switch from head-parallel attention
to sequence-parallel output projection.

Per core i (4 heads = 512 features):
  phase Q/K : xq^T = wq_i @ x^T   (feature-major [512, 4096]), fused RoPE via a
              pair-swap permutation matmul + DVE combine with cos/sin tables.
  phase V   : v = x @ wv_i^T      (token-major [4096, 512]).
  attention : per (batch, head): transposed scores via K-stationary matmuls,
              masked exp on ScalarE, denominator via a ones-matmul chain,
              PV matmul chain, normalize by 1/denom (K=1 ones matmul bcast).
  AllToAll  : per batch, head-slices -> token-slices across the 8 cores.
  phase WO  : out[tok_slice] = attn[tok_slice] @ wo^T, streamed per batch so
              batch-0 WO overlaps batch-1 attention and the second AllToAll.

All PE-facing tensors are float32r (FP22 truncation in the PE, full speed for
moving dims >= 256). Host pre-tiles x/wo/cos/sin so every DMA is contiguous.
"""

import sys

if "/opt/trn_rl_repo" not in sys.path:
    sys.path.insert(0, "/opt/trn_rl_repo")

import numpy as np

import concourse.bass as bass
import concourse.tile as tile
from concourse import bacc, mybir
from concourse.bass_utils import run_bass_kernel_spmd

F32 = mybir.dt.float32
F32R = mybir.dt.float32r

B, S, D = 2, 2048, 4096
H, HD = 32, 128
NCORES = 8
HPC = H // NCORES        # heads per core
F = HPC * HD             # 512 features per core
TOK = B * S              # 4096 tokens
KT = D // 128            # 32 contraction tiles
NB = TOK // 256          # 16 token blocks of 256
SCALE = 1.0 / float(np.sqrt(HD))
NEG = -1e30

_CACHE = {}


def _build():
    nc = bacc.Bacc("TRN2", target_bir_lowering=False, debug=False,
                   num_devices=NCORES)

    # host-tiled inputs: every leaf is a contiguous DMA chunk
    x_d = nc.dram_tensor("xt", [NB, KT, 128, 256], F32R, kind="ExternalInput")
    wq_d = nc.dram_tensor("wqT", [KT, 128, F], F32R, kind="ExternalInput")
    wk_d = nc.dram_tensor("wkT", [KT, 128, F], F32R, kind="ExternalInput")
    wv_d = nc.dram_tensor("wvT", [KT, 128, F], F32R, kind="ExternalInput")
    wo_d = nc.dram_tensor("woT", [D // 512, 2, 16, 128, 512], F32R,
                          kind="ExternalInput")
    cos_d = nc.dram_tensor("cosE", [S // 256, 128, 256], F32, kind="ExternalInput")
    sin_d = nc.dram_tensor("sinE", [S // 256, 128, 256], F32, kind="ExternalInput")
    mask_d = nc.dram_tensor("masks", [128, 4 * 512], F32, kind="ExternalInput")
    perm_d = nc.dram_tensor("permT", [128, 128], F32R, kind="ExternalInput")
    ones_d = nc.dram_tensor("ones", [128, 128], F32R, kind="ExternalInput")
    out_d = nc.dram_tensor("out", [TOK // NCORES, D], F32, kind="ExternalOutput")

    with tile.TileContext(nc) as tc:
        dram = tc.alloc_tile_pool(name="dram", bufs=1, space="DRAM")
        q_sp = dram.tile([NB, HPC, 128, 256], F32R, name="q_sp")
        k_sp = dram.tile([NB, HPC, 128, 256], F32R, name="k_sp")
        v_sp = dram.tile([B, HPC, S // 128, 128, 128], F32R, name="v_sp")
        a2a_in = [dram.tile([NCORES, F, 256], F32R, name=f"a2a_in{b}")
                  for b in range(B)]
        a2a_out = [dram.tile([NCORES, F, 256], F32R, name=f"a2a_out{b}")
                   for b in range(B)]

        with tc.tile_pool(name="consts", bufs=1) as cpool:
            perm_sb = cpool.tile([128, 128], F32R)
            nc.sync.dma_start(out=perm_sb[:], in_=perm_d[:, :])
            ones_sb = cpool.tile([128, 128], F32R)
            nc.sync.dma_start(out=ones_sb[:], in_=ones_d[:, :])

            # ======== projection phases: merged Q+K pass (one pass over x,
            # wq+wk resident), then V pass reusing the weight slots
            with tc.tile_pool(name="wpool", bufs=64) as wpool, \
                 tc.tile_pool(name="xpool", bufs=3) as xpool, \
                 tc.tile_pool(name="cspool", bufs=2) as cspool, \
                 tc.tile_pool(name="prps", bufs=2, space="PSUM") as prps, \
                 tc.tile_pool(name="rotps", bufs=2, space="PSUM") as rotps, \
                 tc.tile_pool(name="ropew", bufs=2) as work:

                def load_w(w_d):
                    tiles = []
                    for kt in range(KT):
                        t = wpool.tile([128, F], F32R, tag="w", name="w_t")
                        nc.sync.dma_start(out=t[:], in_=w_d[kt, :, :])
                        tiles.append(t)
                    return tiles

                def load_x_half(nb, half):
                    xh = xpool.tile([128, 16 * 256], F32R, tag="xh", name="xh")
                    for kk in range(16):
                        eng = nc.gpsimd
                        eng.dma_start(
                            out=xh[:, kk * 256:(kk + 1) * 256],
                            in_=x_d[nb, half * 16 + kk, :, :])
                    return xh

                # ---- merged phase Q+K (feature-major + RoPE)
                wq_t = load_w(wq_d)
                wk_t = load_w(wk_d)
                for nb in range(NB):
                    xh = [load_x_half(nb, 0), load_x_half(nb, 1)]
                    sb_idx = nb % (S // 256)
                    cos_sb = cspool.tile([128, 256], F32, tag="cos",
                                         name="cos_sb")
                    nc.sync.dma_start(out=cos_sb[:], in_=cos_d[sb_idx, :, :])
                    sin_sb = cspool.tile([128, 256], F32, tag="sin",
                                         name="sin_sb")
                    nc.sync.dma_start(out=sin_sb[:], in_=sin_d[sb_idx, :, :])
                    for m in range(2 * HPC):
                        w_t = wq_t if m < HPC else wk_t
                        o_sp = q_sp if m < HPC else k_sp
                        mm = m % HPC
                        ps = prps.tile([128, 256], F32, name="ps")
                        for half in range(2):
                            for kk in range(16):
                                kt = half * 16 + kk
                                nc.tensor.matmul(
                                    ps[:],
                                    w_t[kt][:, mm * 128:(mm + 1) * 128],
                                    xh[half][:, kk * 256:(kk + 1) * 256],
                                    start=(kt == 0), stop=(kt == KT - 1))
                        raw = work.tile([128, 256], F32R, tag="raw",
                                        name="raw")
                        nc.scalar.copy(raw[:], ps[:])
                        rot = rotps.tile([128, 256], F32, name="rot")
                        nc.tensor.matmul(rot[:], perm_sb[:], raw[:],
                                         start=True, stop=True)
                        t1 = work.tile([128, 256], F32, tag="t1", name="t1")
                        nc.vector.tensor_mul(t1[:], raw[:], cos_sb[:])
                        t2 = work.tile([128, 256], F32, tag="t2", name="t2")
                        nc.vector.tensor_mul(t2[:], rot[:], sin_sb[:])
                        qf = work.tile([128, 256], F32R, tag="qf", name="qf")
                        nc.vector.tensor_add(qf[:], t1[:], t2[:])
                        nc.sync.dma_start(out=o_sp[nb, mm, :, :], in_=qf[:])

                # ---- phase V (token-major)
                wv_t = load_w(wv_d)
                for nb in range(NB):
                    xh = [load_x_half(nb, 0), load_x_half(nb, 1)]
                    for mt in range(2):
                        ps = prps.tile([128, F], F32, name="psv", tag="psv")
                        for half in range(2):
                            for kk in range(16):
                                kt = half * 16 + kk
                                nc.tensor.matmul(
                                    ps[:],
                                    xh[half][:, kk * 256 + mt * 128:
                                             kk * 256 + (mt + 1) * 128],
                                    wv_t[kt][:],
                                    start=(kt == 0), stop=(kt == KT - 1))
                        v_sb = work.tile([128, F], F32R, tag="vsb", name="v_sb")
                        nc.scalar.copy(v_sb[:], ps[:])
                        tok0 = nb * 256 + mt * 128
                        b, st = tok0 // S, (tok0 % S) // 128
                        for h in range(HPC):
                            nc.sync.dma_start(
                                out=v_sp[b, h, st, :, :],
                                in_=v_sb[:, h * 128:(h + 1) * 128])

            # ======== attention + per-batch AllToAll + per-batch WO
            with tc.tile_pool(name="aqkv", bufs=2) as apool, \
                 tc.tile_pool(name="exw", bufs=28) as expool, \
                 tc.tile_pool(name="amisc", bufs=2) as misc, \
                 tc.tile_pool(name="scps", bufs=3, space="PSUM") as scps, \
                 tc.tile_pool(name="pvps", bufs=1, space="PSUM") as pvps, \
                 tc.tile_pool(name="dps", bufs=2, space="PSUM") as dps, \
                 tc.tile_pool(name="atp", bufs=1) as atpool, \
                 tc.tile_pool(name="wop", bufs=2) as wopool, \
                 tc.tile_pool(name="pswo", bufs=1, space="PSUM") as wops, \
                 tc.tile_pool(name="wout", bufs=3) as wout:

                mask_sb = misc.tile([128, 4 * 512], F32, tag="mask", bufs=1,
                                    name="mask_sb")
                nc.sync.dma_start(out=mask_sb[:], in_=mask_d[:, :])
                for b in range(B):
                    # ---- attention for this batch's 4 heads
                    for h in range(HPC):
                        q_sb = apool.tile([128, S], F32R, tag="q", name="q_sb")
                        k_sb = apool.tile([128, S], F32R, tag="k", name="k_sb")
                        for j in range(S // 256):
                            nc.sync.dma_start(
                                out=q_sb[:, j * 256:(j + 1) * 256],
                                in_=q_sp[b * (S // 256) + j, h, :, :])
                            nc.sync.dma_start(
                                out=k_sb[:, j * 256:(j + 1) * 256],
                                in_=k_sp[b * (S // 256) + j, h, :, :])
                        v_sb = apool.tile([128, S], F32R, tag="v", name="v_sb")
                        for st in range(S // 128):
                            nc.sync.dma_start(
                                out=v_sb[:, st * 128:(st + 1) * 128],
                                in_=v_sp[b, h, st, :, :])
                        def sc_chain(qt):
                            nkt = 4 * qt + 4
                            exs = []
                            for kt in range(nkt):
                                sc = scps.tile([128, 512], F32, name="sc")
                                nc.tensor.matmul(
                                    sc[:], k_sb[:, kt * 128:(kt + 1) * 128],
                                    q_sb[:, qt * 512:(qt + 1) * 512],
                                    start=True, stop=True)
                                r = kt - 4 * qt
                                ex = expool.tile([128, 512], F32R, tag="ex",
                                                 name="ex")
                                if r >= 0:
                                    scm = expool.tile([128, 512], F32, bufs=3,
                                                      tag="scm", name="scm")
                                    nc.vector.tensor_add(
                                        scm[:], sc[:],
                                        mask_sb[:, r * 512:(r + 1) * 512])
                                    src_t = scm
                                else:
                                    src_t = sc
                                nc.scalar.activation(
                                    ex[:], src_t[:],
                                    mybir.ActivationFunctionType.Exp,
                                    scale=SCALE)
                                exs.append(ex)
                            return exs

                        def finish(qt, exs):
                            nkt = 4 * qt + 4
                            dsum = dps.tile([1, 512], F32, name="dsum",
                                            tag="dsum")
                            for kt in range(nkt):
                                nc.tensor.matmul(
                                    dsum[:], ones_sb[:, 0:1], exs[kt][:],
                                    start=(kt == 0), stop=(kt == nkt - 1))
                            rec = misc.tile([1, 512], F32R, tag="rec",
                                            name="rec")
                            with nc.allow_low_precision(
                                    reason="1/denom consumed by f32r matmul"):
                                nc.vector.reciprocal(rec[:], dsum[:])
                            pv = pvps.tile([128, 512], F32, name="pv")
                            for kt in range(nkt):
                                nc.tensor.matmul(
                                    pv[:], v_sb[:, kt * 128:(kt + 1) * 128],
                                    exs[kt][:],
                                    start=(kt == 0), stop=(kt == nkt - 1))
                            bc = dps.tile([128, 512], F32, name="bc",
                                          tag="dsum")
                            nc.tensor.matmul(bc[:], ones_sb[0:1, :], rec[:],
                                             start=True, stop=True)
                            bc_sb = misc.tile([128, 512], F32, tag="bcsb",
                                              name="bc_sb")
                            nc.vector.tensor_copy(bc_sb[:], bc[:])
                            at = misc.tile([128, 512], F32R, tag="at",
                                           name="at")
                            nc.vector.tensor_mul(at[:], pv[:], bc_sb[:])
                            for u in range(2):
                                nc.sync.dma_start(
                                    out=a2a_in[b][2 * qt + u,
                                                  h * 128:(h + 1) * 128, :],
                                    in_=at[:, u * 256:(u + 1) * 256])

                        pending = None
                        for qt in range(4):
                            exs = sc_chain(qt)
                            if pending is not None:
                                finish(*pending)
                            pending = (qt, exs)
                        finish(*pending)
                    nc.gpsimd.collective_compute(
                        "AllToAll", mybir.AluOpType.bypass,
                        replica_groups=[list(range(NCORES))],
                        ins=[a2a_in[b][:]], outs=[a2a_out[b][:]])

                # ---- single WO pass: batch 0 first (overlaps the second
                # AllToAll), then batch 1
                for b in range(B):
                    at_sb = atpool.tile([128, KT * 256], F32R, tag="atsb",
                                        name="at_sb")
                    for kt in range(KT):
                        jj, off = (kt * 128) // F, (kt * 128) % F
                        nc.sync.dma_start(
                            out=at_sb[:, kt * 256:(kt + 1) * 256],
                            in_=a2a_out[b][jj, off:off + 128, :])
                    for n in range(D // 512):
                        pss = [wops.tile([128, 512], F32, tag=f"pw{mt}",
                                         name=f"ps{mt}") for mt in range(2)]
                        for quad in range(4):
                            wo_sb = wopool.tile([128, 8 * 512], F32R,
                                                tag="wo", name="wo_sb")
                            for kk in range(8):
                                kt = quad * 8 + kk
                                nc.gpsimd.dma_start(
                                    out=wo_sb[:, kk * 512:(kk + 1) * 512],
                                    in_=wo_d[n, kt // 16, kt % 16, :, :])
                            for mt in range(2):
                                for kk in range(8):
                                    kt = quad * 8 + kk
                                    nc.tensor.matmul(
                                        pss[mt][:],
                                        at_sb[:, kt * 256 + mt * 128:
                                              kt * 256 + (mt + 1) * 128],
                                        wo_sb[:, kk * 512:(kk + 1) * 512],
                                        start=(kt == 0), stop=(kt == KT - 1))
                        for mt in range(2):
                            o_sb = wout.tile([128, 512], F32, name="o_sb")
                            nc.scalar.copy(o_sb[:], pss[mt][:])
                            nc.sync.dma_start(
                                out=out_d[b * 256 + mt * 128:
                                          b * 256 + (mt + 1) * 128,
                                          n * 512:(n + 1) * 512],
                                in_=o_sb[:])

    nc.compile()
    return nc


def _host_inputs(x, wq, wk, wv, wo):
    x = np.asarray(x, dtype=np.float32)
    xT = np.ascontiguousarray(x.reshape(TOK, D).T)            # [D, TOK]
    xt = np.ascontiguousarray(
        xT.reshape(KT, 128, NB, 256).transpose(2, 0, 1, 3))   # [NB,KT,128,256]
    woT = np.asarray(wo, dtype=np.float32).T                  # [f, d_out]
    wot = np.ascontiguousarray(
        woT.reshape(KT, 128, D // 512, 512).transpose(2, 0, 1, 3)
        .reshape(D // 512, 2, 16, 128, 512))

    inv = (1.0 / (10000.0 ** (np.arange(0, HD, 2, dtype=np.float64) / HD)))
    fr = np.outer(np.arange(S, dtype=np.float64), inv)        # [S, HD/2]
    cosE = np.repeat(np.cos(fr).T, 2, axis=0).astype(np.float32)  # [128, S]
    sinE = np.repeat(np.sin(fr).T, 2, axis=0).astype(np.float32)
    cost = np.ascontiguousarray(
        cosE.reshape(128, S // 256, 256).transpose(1, 0, 2))
    sint = np.ascontiguousarray(
        sinE.reshape(128, S // 256, 256).transpose(1, 0, 2))

    masks = np.zeros([128, 4 * 512], dtype=np.float32)
    qi = np.arange(512)
    pi = np.arange(128)
    for r in range(4):
        masks[:, r * 512:(r + 1) * 512][qi[None, :] < (r * 128 + pi)[:, None]] = NEG

    permT = np.zeros([128, 128], dtype=np.float32)
    ii = np.arange(0, 128, 2)
    permT[ii + 1, ii] = -1.0
    permT[ii, ii + 1] = 1.0

    ones = np.ones([128, 128], dtype=np.float32)

    maps = []
    for i in range(NCORES):
        sl = slice(i * F, (i + 1) * F)
        maps.append(dict(
            xt=xt,
            wqT=np.ascontiguousarray(
                np.asarray(wq, np.float32)[sl, :].T.reshape(KT, 128, F)),
            wkT=np.ascontiguousarray(
                np.asarray(wk, np.float32)[sl, :].T.reshape(KT, 128, F)),
            wvT=np.ascontiguousarray(
                np.asarray(wv, np.float32)[sl, :].T.reshape(KT, 128, F)),
            woT=wot,
            cosE=cost, sinE=sint, masks=masks, permT=permT, ones=ones,
        ))
    return maps


def kernel(x, start_pos, wq, wk, wv, wo, _trace=False):
    if "nc" not in _CACHE:
        _CACHE["nc"] = _build()
    nc = _CACHE["nc"]
    maps = _host_inputs(x, wq, wk, wv, wo)
    res = run_bass_kernel_spmd(nc, maps, core_ids=list(range(NCORES)),
                               trace=_trace)
    _CACHE["last"] = res
    full = np.empty([TOK, D], dtype=np.float32)
    for j in range(NCORES):
        o = res.results[j]["out"]
        full[j * 256:(j + 1) * 256] = o[:256]
        full[S + j * 256: S + (j + 1) * 256] = o[256:]
    return full.reshape(B, S, D)



# revision 5
# speedup vs baseline: 1.2429x; 1.2429x over previous
"""Trainium2 Bass kernel: causal MHA block (B=2, S=2048, D=4096, 32 heads x 128,
fp32 I/O, interleaved RoPE), tensor-parallel over heads on 8 NeuronCores with a
per-batch AllToAll into a sequence-parallel output projection.

v2 (vs. the fp32r baseline): all PE-facing data is bf16 (same PE rate, half the
HBM bytes and half the DVE/ACT element time), Q/K/V are computed in a single
pass over x with all three weights SBUF-resident, wo is read once (not once per
batch) and prefetched during attention, every DMA is a large merged transfer
(16-32KB per partition row) to kill SWDGE issue overhead, and the causal mask
is a multiplicative 0/1 bf16 mask applied after exp.

Per core i (4 heads = 512 features):
  QKV    : per 512-token block: xq^T/xk^T feature-major chains (RoPE fused via
           pair-swap permutation matmul + DVE combine), v token-major chains.
  attn   : per (batch, head): K-stationary transposed scores, exp on ScalarE
           (PSUM->bf16), denominator via a ones-matmul chain, PV chain,
           normalize with a reciprocal broadcast matmul.
  A2A    : per batch, head-slices -> token-slices across 8 cores (bf16).
  WO     : out[tok_slice] = attn @ wo^T, one pass over wo for both batches.
"""

import sys

if "/opt/trn_rl_repo" not in sys.path:
    sys.path.insert(0, "/opt/trn_rl_repo")

import numpy as np

import concourse.bass as bass
import concourse.tile as tile
from concourse import bacc, mybir
from concourse.bass_utils import run_bass_kernel_spmd

F32 = mybir.dt.float32
BF16 = mybir.dt.bfloat16

B, S, D = 2, 2048, 4096
H, HD = 32, 128
NCORES = 8
HPC = H // NCORES        # heads per core
F = HPC * HD             # 512 features per core
TOK = B * S              # 4096 tokens
KT = D // 128            # 32 contraction tiles
NB = TOK // 512          # 8 token blocks of 512
SCALE = 1.0 / float(np.sqrt(HD))

_CACHE = {}


def _build():
    nc = bacc.Bacc("TRN2", target_bir_lowering=False, debug=False,
                   num_devices=NCORES)

    x_d = nc.dram_tensor("xt", [NB, 2, 128, 16 * 512], BF16,
                         kind="ExternalInput")
    wq_d = nc.dram_tensor("wqT", [128, KT * F], BF16, kind="ExternalInput")
    wk_d = nc.dram_tensor("wkT", [128, KT * F], BF16, kind="ExternalInput")
    wv_d = nc.dram_tensor("wvT", [128, KT * F], BF16, kind="ExternalInput")
    wo_d = nc.dram_tensor("woT", [D // 512, 128, KT * 512], BF16,
                          kind="ExternalInput")
    cos_d = nc.dram_tensor("cosE", [128, S], BF16, kind="ExternalInput")
    sin_d = nc.dram_tensor("sinE", [128, S], BF16, kind="ExternalInput")
    tri_d = nc.dram_tensor("tri01", [128, 4 * 512], BF16, kind="ExternalInput")
    perm_d = nc.dram_tensor("permT", [128, 128], BF16, kind="ExternalInput")
    ones_d = nc.dram_tensor("ones", [128, 128], BF16, kind="ExternalInput")
    out_d = nc.dram_tensor("out", [TOK // NCORES, D], F32,
                           kind="ExternalOutput")

    with tile.TileContext(nc) as tc:
        dram = tc.alloc_tile_pool(name="dram", bufs=1, space="DRAM")
        q_sp = dram.tile([HPC, 128, TOK], BF16, name="q_sp")
        k_sp = dram.tile([HPC, 128, TOK], BF16, name="k_sp")
        v_sp = dram.tile([B, 128, (S // 128) * F], BF16, name="v_sp")
        a2a_in = [dram.tile([NCORES, F, 256], BF16, name=f"a2a_in{b}")
                  for b in range(B)]
        a2a_out = [dram.tile([NCORES, F, 256], BF16, name=f"a2a_out{b}")
                   for b in range(B)]

        with tc.tile_pool(name="consts", bufs=1) as cpool:
            perm_sb = cpool.tile([128, 128], BF16)
            nc.sync.dma_start(out=perm_sb[:], in_=perm_d[:, :])
            ones_sb = cpool.tile([128, 128], BF16)
            nc.sync.dma_start(out=ones_sb[:], in_=ones_d[:, :])
            cos_sb = cpool.tile([128, S], BF16)
            nc.sync.dma_start(out=cos_sb[:], in_=cos_d[:, :])
            sin_sb = cpool.tile([128, S], BF16)
            nc.sync.dma_start(out=sin_sb[:], in_=sin_d[:, :])
            tri_sb = cpool.tile([128, 4 * 512], BF16)
            nc.sync.dma_start(out=tri_sb[:], in_=tri_d[:, :])

            # ======== single pass over x: Q, K (feature-major + RoPE) and V
            # (token-major), all three weights SBUF-resident in bf16
            with tc.tile_pool(name="wpool", bufs=1) as wpool, \
                 tc.tile_pool(name="xpool", bufs=2) as xpool, \
                 tc.tile_pool(name="qkvw", bufs=2) as work, \
                 tc.tile_pool(name="prps", bufs=2, space="PSUM") as prps, \
                 tc.tile_pool(name="rotps", bufs=2, space="PSUM") as rotps:

                w_sb = {}
                for nm, w_d in (("q", wq_d), ("k", wk_d), ("v", wv_d)):
                    t = wpool.tile([128, KT * F], BF16, tag=f"w{nm}",
                                   name=f"w{nm}")
                    nc.sync.dma_start(out=t[:], in_=w_d[:, :])
                    w_sb[nm] = t

                for nb in range(NB):
                    xh = xpool.tile([128, KT * 512], BF16, tag="xh", name="xh")
                    for half in range(2):
                        nc.gpsimd.dma_start(
                            out=xh[:, half * 8192:(half + 1) * 8192],
                            in_=x_d[nb, half, :, :])
                    pos = (nb % (S // 512)) * 512
                    for m in range(2 * HPC):
                        wt = w_sb["q"] if m < HPC else w_sb["k"]
                        o_sp = q_sp if m < HPC else k_sp
                        h = m % HPC
                        ps = prps.tile([128, 512], F32, name="ps")
                        for kt in range(KT):
                            nc.tensor.matmul(
                                ps[:],
                                wt[:, kt * F + h * 128: kt * F + (h + 1) * 128],
                                xh[:, kt * 512:(kt + 1) * 512],
                                start=(kt == 0), stop=(kt == KT - 1))
                        raw = work.tile([128, 512], BF16, tag="raw",
                                        name="raw")
                        nc.scalar.copy(raw[:], ps[:])
                        rot = rotps.tile([128, 512], F32, name="rot")
                        nc.tensor.matmul(rot[:], perm_sb[:], raw[:],
                                         start=True, stop=True)
                        t1 = work.tile([128, 512], F32, tag="t1", name="t1")
                        nc.vector.tensor_mul(t1[:], raw[:],
                                             cos_sb[:, pos:pos + 512])
                        t2 = work.tile([128, 512], F32, tag="t2", name="t2")
                        nc.vector.tensor_mul(t2[:], rot[:],
                                             sin_sb[:, pos:pos + 512])
                        qf = work.tile([128, 512], BF16, tag="qf", name="qf")
                        nc.vector.tensor_add(qf[:], t1[:], t2[:])
                        nc.sync.dma_start(
                            out=o_sp[h, :, nb * 512:(nb + 1) * 512],
                            in_=qf[:])
                    for ts in range(4):
                        ps = prps.tile([128, 512], F32, name="psv")
                        for kt in range(KT):
                            nc.tensor.matmul(
                                ps[:],
                                xh[:, kt * 512 + ts * 128:
                                   kt * 512 + (ts + 1) * 128],
                                w_sb["v"][:, kt * F:(kt + 1) * F],
                                start=(kt == 0), stop=(kt == KT - 1))
                        vf = work.tile([128, 512], BF16, tag="vf", name="vf")
                        nc.scalar.copy(vf[:], ps[:])
                        st_g = nb * 4 + ts
                        nc.sync.dma_start(
                            out=v_sp[st_g // 16, :,
                                     (st_g % 16) * F:(st_g % 16 + 1) * F],
                            in_=vf[:])

            # ======== attention + per-batch AllToAll, then single-pass WO
            with tc.tile_pool(name="aqk", bufs=2) as apool, \
                 tc.tile_pool(name="avp", bufs=1) as vpool, \
                 tc.tile_pool(name="exw", bufs=26) as expool, \
                 tc.tile_pool(name="amisc", bufs=2) as misc, \
                 tc.tile_pool(name="atsb", bufs=1) as atsb, \
                 tc.tile_pool(name="wop", bufs=2) as wopool, \
                 tc.tile_pool(name="wout", bufs=3) as wout:

                with tc.tile_pool(name="scps", bufs=3, space="PSUM") as scps, \
                     tc.tile_pool(name="pvps", bufs=2, space="PSUM") as pvps, \
                     tc.tile_pool(name="dps", bufs=2, space="PSUM") as dps, \
                     tc.tile_pool(name="bcps", bufs=1, space="PSUM") as bcps:

                    for b in range(B):
                        v_sb = vpool.tile([128, (S // 128) * F], BF16,
                                          tag="v", name="v_sb")
                        nc.sync.dma_start(out=v_sb[:], in_=v_sp[b, :, :])
                        for h in range(HPC):
                            q_sb = apool.tile([128, S], BF16, tag="q",
                                              name="q_sb")
                            nc.sync.dma_start(
                                out=q_sb[:], in_=q_sp[h, :, b * S:(b + 1) * S])
                            k_sb = apool.tile([128, S], BF16, tag="k",
                                              name="k_sb")
                            nc.sync.dma_start(
                                out=k_sb[:], in_=k_sp[h, :, b * S:(b + 1) * S])

                            def sc_chain(qt):
                                nkt = 4 * qt + 4
                                exs = []
                                for kt in range(nkt):
                                    sc = scps.tile([128, 512], F32, name="sc")
                                    nc.tensor.matmul(
                                        sc[:],
                                        k_sb[:, kt * 128:(kt + 1) * 128],
                                        q_sb[:, qt * 512:(qt + 1) * 512],
                                        start=True, stop=True)
                                    ex = expool.tile([128, 512], BF16,
                                                     tag="ex", name="ex")
                                    nc.scalar.activation(
                                        ex[:], sc[:],
                                        mybir.ActivationFunctionType.Exp,
                                        scale=SCALE)
                                    r = kt - 4 * qt
                                    if r >= 0:
                                        exm = expool.tile(
                                            [128, 512], BF16, tag="ex",
                                            name="exm")
                                        nc.vector.tensor_mul(
                                            exm[:], ex[:],
                                            tri_sb[:, r * 512:(r + 1) * 512])
                                        ex = exm
                                    exs.append(ex)
                                return exs

                            def finish(qt, exs):
                                nkt = 4 * qt + 4
                                dsum = dps.tile([1, 512], F32, name="dsum",
                                                tag="dsum")
                                for kt in range(nkt):
                                    nc.tensor.matmul(
                                        dsum[:], ones_sb[:, 0:1], exs[kt][:],
                                        start=(kt == 0), stop=(kt == nkt - 1))
                                rec = misc.tile([1, 512], BF16, tag="rec",
                                                name="rec")
                                with nc.allow_low_precision(
                                        reason="1/denom feeds bf16 matmul"):
                                    nc.vector.reciprocal(rec[:], dsum[:])
                                pv = pvps.tile([128, 512], F32, name="pv")
                                for kt in range(nkt):
                                    nc.tensor.matmul(
                                        pv[:],
                                        v_sb[:, kt * F + h * 128:
                                             kt * F + (h + 1) * 128],
                                        exs[kt][:],
                                        start=(kt == 0), stop=(kt == nkt - 1))
                                bc = bcps.tile([128, 512], F32, name="bc")
                                nc.tensor.matmul(bc[:], ones_sb[0:1, :],
                                                 rec[:], start=True, stop=True)
                                bc_sb = misc.tile([128, 512], BF16,
                                                  tag="bcsb", name="bc_sb")
                                nc.vector.tensor_copy(bc_sb[:], bc[:])
                                at = misc.tile([128, 512], BF16, tag="at",
                                               name="at")
                                nc.vector.tensor_mul(at[:], pv[:], bc_sb[:])
                                for u in range(2):
                                    nc.sync.dma_start(
                                        out=a2a_in[b][2 * qt + u,
                                                      h * 128:(h + 1) * 128,
                                                      :],
                                        in_=at[:, u * 256:(u + 1) * 256])

                            pending = None
                            for qt in range(4):
                                exs = sc_chain(qt)
                                if pending is not None:
                                    finish(*pending)
                                pending = (qt, exs)
                            finish(*pending)
                        nc.gpsimd.collective_compute(
                            "AllToAll", mybir.AluOpType.bypass,
                            replica_groups=[list(range(NCORES))],
                            ins=[a2a_in[b][:]], outs=[a2a_out[b][:]])

                # ---- WO: one pass over wo; wo loads (sync) prefetch during
                # batch-1 attention; at_sb loads go on the vector queue so the
                # a2a_out[1] dependency never blocks the wo stream.
                with tc.tile_pool(name="pswo", bufs=1, space="PSUM") as wops:
                    at_sb = []
                    for b in range(B):
                        t = atsb.tile([128, KT * 256], BF16, tag=f"at{b}",
                                      name=f"at_sb{b}")
                        for kt in range(KT):
                            nc.scalar.dma_start(
                                out=t[:, kt * 256:(kt + 1) * 256],
                                in_=a2a_out[b][kt // 4,
                                               (kt % 4) * 128:
                                               (kt % 4 + 1) * 128, :])
                        at_sb.append(t)
                    for n in range(D // 512):
                        wo_sb = wopool.tile([128, KT * 512], BF16, tag="wo",
                                            name="wo_sb")
                        nc.sync.dma_start(out=wo_sb[:], in_=wo_d[n, :, :])
                        pss = [[wops.tile([128, 512], F32, tag=f"pw{b}{mt}",
                                          name=f"ps{b}{mt}")
                                for mt in range(2)] for b in range(B)]
                        for kt in range(KT):
                            for b in range(B):
                                for mt in range(2):
                                    nc.tensor.matmul(
                                        pss[b][mt][:],
                                        at_sb[b][:, kt * 256 + mt * 128:
                                                 kt * 256 + (mt + 1) * 128],
                                        wo_sb[:, kt * 512:(kt + 1) * 512],
                                        start=(kt == 0), stop=(kt == KT - 1))
                        for b in range(B):
                            for mt in range(2):
                                o_sb = wout.tile([128, 512], F32, name="o_sb")
                                nc.scalar.copy(o_sb[:], pss[b][mt][:])
                                nc.scalar.dma_start(
                                    out=out_d[b * 256 + mt * 128:
                                              b * 256 + (mt + 1) * 128,
                                              n * 512:(n + 1) * 512],
                                    in_=o_sb[:])

    nc.compile()
    return nc


def _host_inputs(x, wq, wk, wv, wo):
    import ml_dtypes
    BF = ml_dtypes.bfloat16

    x = np.asarray(x, dtype=np.float32).reshape(TOK, D)
    # xt[nb, half, p, kk*512+t] = x[nb*512+t, half*2048+kk*128+p]
    xt = np.ascontiguousarray(
        x.T.reshape(2, 16, 128, NB, 512).transpose(3, 0, 2, 1, 4)
        .reshape(NB, 2, 128, 16 * 512)).astype(BF)

    # woT[n, p, kt*512+o] = wo[n*512+o, kt*128+p]
    wot = np.ascontiguousarray(
        np.asarray(wo, np.float32).T.reshape(KT, 128, D // 512, 512)
        .transpose(2, 1, 0, 3).reshape(D // 512, 128, KT * 512)).astype(BF)

    inv = (1.0 / (10000.0 ** (np.arange(0, HD, 2, dtype=np.float64) / HD)))
    fr = np.outer(np.arange(S, dtype=np.float64), inv)        # [S, HD/2]
    cosE = np.repeat(np.cos(fr).T, 2, axis=0).astype(BF)      # [128, S]
    sinE = np.repeat(np.sin(fr).T, 2, axis=0).astype(BF)

    # tri01[p, r*512+q] = 1 where r*128+p <= q (causal keep), else 0
    tri = np.zeros([128, 4 * 512], dtype=np.float32)
    qi = np.arange(512)
    pi = np.arange(128)
    for r in range(4):
        tri[:, r * 512:(r + 1) * 512][
            (r * 128 + pi)[:, None] <= qi[None, :]] = 1.0
    tri = tri.astype(BF)

    permT = np.zeros([128, 128], dtype=np.float32)
    ii = np.arange(0, 128, 2)
    permT[ii + 1, ii] = -1.0
    permT[ii, ii + 1] = 1.0
    permT = permT.astype(BF)

    ones = np.ones([128, 128], dtype=BF)

    def wtile(w, i):
        # [p, kt*512+f] = w[i*512+f, kt*128+p]
        sl = np.asarray(w, np.float32)[i * F:(i + 1) * F, :]
        return np.ascontiguousarray(
            sl.T.reshape(KT, 128, F).transpose(1, 0, 2)
            .reshape(128, KT * F)).astype(BF)

    maps = []
    for i in range(NCORES):
        maps.append(dict(
            xt=xt,
            wqT=wtile(wq, i), wkT=wtile(wk, i), wvT=wtile(wv, i),
            woT=wot, cosE=cosE, sinE=sinE, tri01=tri, permT=permT,
            ones=ones,
        ))
    return maps


def kernel(x, start_pos, wq, wk, wv, wo, _trace=False):
    if "nc" not in _CACHE:
        _CACHE["nc"] = _build()
    nc = _CACHE["nc"]
    maps = _host_inputs(x, wq, wk, wv, wo)
    res = run_bass_kernel_spmd(nc, maps, core_ids=list(range(NCORES)),
                               trace=_trace)
    _CACHE["last"] = res
    full = np.empty([TOK, D], dtype=np.float32)
    for j in range(NCORES):
        o = res.results[j]["out"]
        full[j * 256:(j + 1) * 256] = o[:256]
        full[S + j * 256: S + (j + 1) * 256] = o[256:]
    return full.reshape(B, S, D)
